# revision 21
# baseline (speedup 1.0000x reference)
"""MinGRU forward on 8 Trainium2 NeuronCores.

Reference computation (per batch b):
    k       = x @ Wz + bz                 # [T, H]
    z       = sigmoid(k)
    c       = 1 - z
    htilde  = g(x @ Wh + bh)              # g(a) = a+0.5 if a>=0 else sigmoid(a)
                                          #      = max(a+0.5, sigmoid(a))
    h[0]    = g(h_0)
    h[t]    = c[t-1]*h[t-1] + z[t-1]*htilde[t-1]   (t = 1..T)
    out     = h                           # [T+1, H]

The log-space cumlogsumexp in the reference is exactly this linear
recurrence (all quantities positive, coefficients in (0,1), so the
linear form is numerically stable).

Sharding: data-parallel over batch, one batch per core, weights
replicated.

The kernel is Tensor-engine bound: 1024 fp16 matmuls/core = 218.5us at
2.4GHz. fp8 DoubleRow was measured on hardware at ~1 cycle/output-row
(2x FLOPs via 256-deep contraction, not the cost model's 4x), so
error-compensated hi/lo fp8 (3 logical matmuls, verified numerically
at 0.008 max rel err) is 1.5x SLOWER than fp16 — fp16 is optimal.
The optimization is therefore all PE-occupancy at the edges
(257.7us -> 245.1us measured):
  - x is transposed AND cast to fp16 on the host, so the device issues
    only plain contiguous DMAs. The baseline's device-side DMA-transpose
    serialized the weight loads behind it, costing ~8us of PE idle at
    kernel start.
  - DMA priority: weights stream on the in-order sync ring ordered
    exactly as chunk 0's k-outer schedule consumes them (all low
    m-halves first, wh0's low half split so the first matmul waits on
    32KB); later x chunks queue BEHIND the weights on the same ring
    (they are not needed for ~28us, and racing the weights stalled the
    PE ~8us). Chunk 0's x rides the GpSimd ring k-slice by k-slice,
    its first slice and the packed constants the near-empty ACT ring.
    Small constants are host-packed into one [128, 24] block — the
    natural per-vector rearranges emit 1024 4-byte scatter descriptors
    that starve the critical first weight slice.
  - Gates run fp16 end to end (z, s, c, g, v, h): DVE gets 2x
    throughput on 16-bit SBUF operands, the output DMA halves, and ACT
    drops from 3 sigmoids to 2 (c = 1-z moves to a cheap DVE
    tensor_scalar). GpSimd compute is not used at all (its software
    multiply is ~4x slower than DVE fp16).
  - The scan keeps fp32 state internally (hardware guarantee) and only
    stores h as fp16; rel err stays ~4.6e-3 (limit 2e-2).
  - The last 512 timesteps run as two 256 chunks and the final tile's
    gates in two 128 slices, so the post-matmul tail chain is short.
  - No PE warm-up tricks: starting the PE while the DMA streams run
    full-bore trips the HAM power envelope into a 13/16-duty clock
    state (1.95GHz) that persists for the WHOLE run (+38us measured).
The device writes timesteps 1..T transposed ([H, T] fp16); the host
prepends g(h_0), transposes and upcasts during the unshard.
"""

import numpy as np

B, T, D, H = 8, 4096, 1024, 1024
P = 128
TCH = 512                 # time-chunk (one PSUM bank of fp32 per matmul)
KO = D // P               # contraction tiles
MO = H // P               # output-channel tiles
# 7 full chunks + 2 half chunks at the end to shorten the tail
CHUNKS = [(i * TCH, TCH) for i in range(7)] + [(3584, 256), (3840, 256)]
NTCH = T // TCH           # host x layout is uniform 512-chunk-major

_PROGRAM_CACHE = {}


def _build_program():
    import concourse.bacc as bacc
    import concourse.mybir as mybir
    import concourse.tile as tile

    fp32 = mybir.dt.float32
    fp16 = mybir.dt.float16
    SIG = mybir.ActivationFunctionType.Sigmoid
    MUL = mybir.AluOpType.mult
    ADD = mybir.AluOpType.add
    MAX = mybir.AluOpType.max

    nc = bacc.Bacc("TRN2", target_bir_lowering=False)

    # x pre-transposed on host: [ki, nt, ko, t] with D-index = ko*128+ki,
    # T-index = nt*512+t  (chunk-major so each chunk DMA reads 8KB runs)
    xt_ext = nc.declare_dram_parameter("xt", [P, NTCH * KO * TCH], fp16, isOutput=False)
    wz_ext = nc.declare_dram_parameter("Wz", [D, H], fp16, isOutput=False)
    wh_ext = nc.declare_dram_parameter("Wh", [D, H], fp16, isOutput=False)
    # host-packed [bz_t | bh_t | h0_t] in device layout (partition = channel
    # within tile, free = tile): a single small contiguous DMA. The natural
    # per-tensor rearranges generate 1024 4-byte scatter descriptors each,
    # which hogged the DMA engines right when the first weight slice's bulk
    # data needed them.
    cst_ext = nc.declare_dram_parameter("cst", [P, 3 * MO], fp32, isOutput=False)
    # transposed fp16 output, timesteps 1..T; the host prepends g(h_0) and
    # untransposes/upcasts during the gather
    out_ext = nc.declare_dram_parameter("out", [H, T], fp16, isOutput=True)

    xt_r = xt_ext.rearrange("p (nt ko t) -> p nt ko t", nt=NTCH, ko=KO)

    with tile.TileContext(nc) as tc:
        with (
            tc.tile_pool(name="const", bufs=1) as const_pool,
            tc.tile_pool(name="w", bufs=1) as w_pool,
            tc.tile_pool(name="xt", bufs=3) as xt_pool,
            tc.tile_pool(name="ht", bufs=2) as ht_pool,
            tc.tile_pool(name="gate", bufs=3) as gate_pool,
            tc.tile_pool(name="psp", bufs=4, space="PSUM") as psum_p,
        ):
            # Chunk 0's x, k-slice by k-slice on the GpSimd DGE ring (no
            # other traffic), so the first matmul waits only for slice 0.
            # ko=0 (the only slice the first matmul needs) rides the
            # near-empty ACT ring so it isn't starved by the weight stream.
            # The remaining chunk-0 x slices are interleaved INTO the sync
            # ring below in exact consumption order — bursting them on a
            # parallel ring raced the weight stream for DMA bandwidth and
            # stalled the PE ~1.5us in the first 6us.
            xt_first = xt_pool.tile([P, KO, TCH], fp16, tag="xt512", name="xt512")
            nc.scalar.dma_start(xt_first[:, 0], xt_r[:, 0, 0])

            # Weights resident: [ki, ko, h] so lhsT tiles are natural slices.
            # Loaded per k-slice (contiguous 256KB each) on the sync ring, in
            # the order chunk 0's k-outer matmul schedule consumes them
            # (pa/wh first). Later x chunks also ride the sync ring BEHIND
            # the weights: the ring is in-order, so the weight stream gets
            # the DMA bandwidth until it is done (chunk 1 is not needed for
            # ~28us; letting it race the weights stalled the PE ~8us).
            wz_sb = w_pool.tile([P, KO, H], fp16)
            wh_sb = w_pool.tile([P, KO, H], fp16)
            wz_r = wz_ext.rearrange("(ko ki) h -> ki ko h", ki=P)
            wh_r = wh_ext.rearrange("(ko ki) h -> ki ko h", ki=P)
            # Stream order matches chunk 0's k-outer consumption exactly:
            # the first half (m-tiles 0-3) of every k-slice first — wh0's
            # low half further split so the very first matmuls wait on
            # 32KB/96KB — then all high halves. Half 0 of chunk 0 then
            # needs only 2MB of weights in its window instead of 4MB.
            HH = H // 2
            nc.sync.dma_start(wh_sb[:, 0, 0:P], wh_r[:, 0, 0:P])
            nc.sync.dma_start(wh_sb[:, 0, P:HH], wh_r[:, 0, P:HH])
            nc.sync.dma_start(wz_sb[:, 0, 0:HH], wz_r[:, 0, 0:HH])
            for ko in range(1, KO):
                nc.sync.dma_start(xt_first[:, ko], xt_r[:, 0, ko])
                nc.sync.dma_start(wh_sb[:, ko, 0:HH], wh_r[:, ko, 0:HH])
                nc.sync.dma_start(wz_sb[:, ko, 0:HH], wz_r[:, ko, 0:HH])
            for ko in range(KO):
                nc.sync.dma_start(wh_sb[:, ko, HH:], wh_r[:, ko, HH:])
                nc.sync.dma_start(wz_sb[:, ko, HH:], wz_r[:, ko, HH:])

            # Constants in one contiguous DMA on the ACT DGE ring.
            cst_sb = const_pool.tile([P, 3 * MO], fp32)
            nc.scalar.dma_start(cst_sb, cst_ext[:, :])
            bz_sb = cst_sb[:, 0:MO]
            bh_sb = cst_sb[:, MO:2 * MO]
            h0_sb = cst_sb[:, 2 * MO:3 * MO]
            bhp5_sb = const_pool.tile([P, MO], fp32)
            nc.vector.tensor_scalar_add(bhp5_sb, bh_sb, 0.5)

            # g(h_0) for the chunk-0 scan init (out column 0 is host-side)
            s0_sb = const_pool.tile([P, MO], fp32)
            nc.scalar.activation(s0_sb, h0_sb, SIG)
            gh0_sb = const_pool.tile([P, MO], fp32)
            nc.vector.scalar_tensor_tensor(gh0_sb, h0_sb, 0.5, s0_sb, op0=ADD, op1=MAX)

            # x chunks prefetched two ahead on the GpSimd ring. Tiles are
            # tagged by width so the 256-wide tail chunks rotate separately.
            xt_tiles = [xt_first]

            def issue_xt(ci):
                t0, tch = CHUNKS[ci]
                xt_sb = xt_pool.tile(
                    [P, KO, tch], fp16, tag=f"xt{tch}", name=f"xt{tch}"
                )
                nt, toff = divmod(t0, TCH)
                nc.sync.dma_start(xt_sb, xt_r[:, nt, :, toff:toff + tch])
                xt_tiles.append(xt_sb)

            issue_xt(1)

            prev_ht = None  # previous chunk's scan output (carries the state)
            prev_tch = TCH

            def gates_scan_store(m, t0, tch, pk, pa, ht_sb, split=1):
                # split>1 processes the tile in column slices so the very
                # last tile's gate/scan/DMA chain after the final matmul is
                # short. ACT order s -> z: s feeds the longest chain
                # (g -> v -> scan).
                w = tch // split
                for si in range(split):
                    lo, hi = si * w, (si + 1) * w
                    s_sb = gate_pool.tile([P, TCH], fp16, tag="s", name="s")[:, :w]
                    nc.scalar.activation(s_sb, pa[:, lo:hi], SIG, bias=bh_sb[:, m:m + 1])
                    z_sb = gate_pool.tile([P, TCH], fp16, tag="z", name="z")[:, :w]
                    nc.scalar.activation(z_sb, pk[:, lo:hi], SIG, bias=bz_sb[:, m:m + 1])
                    # g = max(a + bh + 0.5, s) straight from PSUM
                    g_sb = gate_pool.tile([P, TCH], fp16, tag="g", name="g")[:, :w]
                    nc.vector.scalar_tensor_tensor(
                        g_sb, pa[:, lo:hi], bhp5_sb[:, m:m + 1], s_sb, op0=ADD, op1=MAX
                    )
                    c_sb = gate_pool.tile([P, TCH], fp16, tag="c", name="c")[:, :w]
                    nc.vector.tensor_scalar(c_sb, z_sb, -1.0, 1.0, op0=MUL, op1=ADD)
                    v_sb = gate_pool.tile([P, TCH], fp16, tag="v", name="v")[:, :w]
                    nc.vector.tensor_mul(v_sb, z_sb, g_sb)

                    init = (
                        gh0_sb[:, m:m + 1]
                        if prev_ht is None
                        else prev_ht[:, m, prev_tch - 1:prev_tch]
                    ) if si == 0 else ht_sb[:, m, lo - 1:lo]
                    nc.vector.tensor_tensor_scan(
                        ht_sb[:, m, lo:hi], c_sb, v_sb, init, op0=MUL, op1=ADD
                    )
                    nc.sync.dma_start(
                        out_ext[m * P:(m + 1) * P, t0 + lo:t0 + hi],
                        ht_sb[:, m, lo:hi],
                    )

            for ci, (t0, tch) in enumerate(CHUNKS):
                if ci + 2 < len(CHUNKS):
                    issue_xt(ci + 2)
                xt_sb = xt_tiles[ci]
                ht_sb = ht_pool.tile([P, MO, TCH], fp16)

                if ci == 0:
                    # k-outer over groups of 4 output tiles: matmuls consume
                    # the weight k-slices in DMA arrival order, so the PE
                    # starts ~15us earlier and never stalls on weight loads
                    # (which would also re-throttle the HAM clock gate).
                    GQ = min(4, MO)
                    for half in range(MO // GQ):
                        pks = [
                            psum_p.tile([P, TCH], fp32, tag="pk", name="pk")
                            for _ in range(GQ)
                        ]
                        pas = [
                            psum_p.tile([P, TCH], fp32, tag="pa", name="pa")
                            for _ in range(GQ)
                        ]
                        # pa first: s/g/v feed the longest downstream chain
                        for ko in range(KO):
                            for q in range(GQ):
                                m = half * GQ + q
                                nc.tensor.matmul(
                                    pas[q],
                                    wh_sb[:, ko, m * P:(m + 1) * P],
                                    xt_sb[:, ko, :],
                                    start=(ko == 0),
                                    stop=(ko == KO - 1),
                                )
                            for q in range(GQ):
                                m = half * GQ + q
                                nc.tensor.matmul(
                                    pks[q],
                                    wz_sb[:, ko, m * P:(m + 1) * P],
                                    xt_sb[:, ko, :],
                                    start=(ko == 0),
                                    stop=(ko == KO - 1),
                                )
                        for q in range(GQ):
                            m = half * GQ + q
                            gates_scan_store(m, t0, tch, pks[q], pas[q], ht_sb)
                else:
                    for m in range(MO):
                        pk = psum_p.tile([P, TCH], fp32, tag="pk", name="pk")[:, :tch]
                        pa = psum_p.tile([P, TCH], fp32, tag="pa", name="pa")[:, :tch]
                        for ko in range(KO):
                            nc.tensor.matmul(
                                pa,
                                wh_sb[:, ko, m * P:(m + 1) * P],
                                xt_sb[:, ko, :tch],
                                start=(ko == 0),
                                stop=(ko == KO - 1),
                            )
                        for ko in range(KO):
                            nc.tensor.matmul(
                                pk,
                                wz_sb[:, ko, m * P:(m + 1) * P],
                                xt_sb[:, ko, :tch],
                                start=(ko == 0),
                                stop=(ko == KO - 1),
                            )
                        last = ci == len(CHUNKS) - 1 and m == MO - 1
                        gates_scan_store(m, t0, tch, pk, pa, ht_sb,
                                         split=2 if last else 1)

                prev_ht = ht_sb
                prev_tch = tch

    nc.finalize()
    return nc


def _get_program():
    if "v4" not in _PROGRAM_CACHE:
        _PROGRAM_CACHE["v4"] = _build_program()
    return _PROGRAM_CACHE["v4"]


def _prep_xt(xb):
    # [T, D] fp32 -> fp16 [ki, nt, ko, tch] with D = ko*128+ki, T = nt*512+tch
    x16 = np.asarray(xb, dtype=np.float16)
    xt = x16.reshape(NTCH, TCH, KO, P).transpose(3, 0, 2, 1)
    return np.ascontiguousarray(xt).reshape(P, NTCH * KO * TCH)


def run(x, h_0, Wz, bz, Wh, bh, trace=False):
    from concourse.bass_utils import run_bass_kernel_spmd

    nc = _get_program()
    wz16 = np.ascontiguousarray(np.asarray(Wz, dtype=np.float16))
    wh16 = np.ascontiguousarray(np.asarray(Wh, dtype=np.float16))
    bz32 = np.asarray(bz, dtype=np.float32)
    bh32 = np.asarray(bh, dtype=np.float32)
    h0_32 = np.asarray(h_0, dtype=np.float32).reshape(B, H)

    def dev_cols(v):  # [H] -> [P, MO] device layout (partition = mi)
        return v.reshape(MO, P).T

    in_maps = [
        {
            "xt": _prep_xt(x[b]),
            "Wz": wz16,
            "Wh": wh16,
            "cst": np.ascontiguousarray(np.concatenate(
                [dev_cols(bz32), dev_cols(bh32), dev_cols(h0_32[b])], axis=1
            )),
        }
        for b in range(B)
    ]
    res = run_bass_kernel_spmd(nc, in_maps, list(range(B)), trace=trace)
    out = np.empty((B, T + 1, H), dtype=np.float32)
    # h[0] = g(h_0) computed on host in fp32
    out[:, 0, :] = np.where(
        h0_32 >= 0.0, h0_32 + 0.5, 1.0 / (1.0 + np.exp(-h0_32))
    )
    for b in range(B):
        out[b, 1:, :] = res.results[b]["out"].T.astype(np.float32)
    return out, res


def kernel(x, h_0, Wz, bz, Wh, bh):
    out, _ = run(x, h_0, Wz, bz, Wh, bh)
    return out
